# revision 1
# baseline (speedup 1.0000x reference)
"""BitLinear fake-quant GEMM on 8 TRN2 NeuronCores.

Reference math:
  abs_mean  = mean(|W|);  thr = 0.7*abs_mean
  Wq        = sign(W) * (|W| >= thr)            (ternary)
  scale_w   = abs_mean / (mean(Wq != 0) + 1e-8)
  sx        = 127 / max(|X|)
  Xq        = round(X * sx)                      (integer valued, |.| <= 127)
  out       = (Xq @ Wq^T) * scale_w / sx

Sharding: data-parallel over tokens (8192/8 = 1024 columns of X^T per core);
W is replicated.  The host hands each core PRE-TRANSPOSED operands (x.T shard
and w.T) so both matmul operands already have the contraction dim
(in_features) on partitions — quantization is elementwise and writes straight
into matmul-ready SBUF layouts; the device performs zero transposes.  The
|x|-max pass reads the transposed shard too, so its last two staging tiles are
still resident when sx arrives and quantize with zero reload.

Stats: each core reduces its own x shard and a distinct 512-row slice of W^T;
one AllGather of the two per-core scalars + local reduce replaces the global
mean/max all-reduces.  The GEMM is exact integer arithmetic: Xq (ints in
[-127,127]) and Wq (in {-1,0,1}) are exactly representable in bf16, and fp32
PSUM accumulation of 4096 products of magnitude <=127 stays below 2^24.  The
nonzero count of Wq falls out of the quantization pass for free via DVE
accum_out side-sums (every core sees the full W, so every core computes the
exact global count).  The final scalar rescale by scale_w/sx is applied on
the host during the unshard, using stats the device emits.

The per-core output is written tile-chunked ([panel][tblock][128][512], each
store one contiguous 256KB run); the host permutes it back during the gather.
"""

from contextlib import ExitStack

import numpy as np

import concourse.bass as bass
import concourse.bass_isa as bass_isa
import concourse.tile as tile
from concourse import bacc, mybir
from concourse.bass import ts as _ts
from concourse.bass_utils import run_bass_kernel_spmd

P = 128
T, I, O = 8192, 4096, 4096  # tokens, in_features, out_features
NC = 8
TSH = T // NC  # 1024 token columns per core
ISL = I // NC  # 512 wT rows per core for stats
NMM = 512  # matmul moving free dim (one fp32 PSUM bank)
GF = 4096  # streaming tile free size (one [128, 4096] fp32 tile = 2 MB)
MAGIC = 12582912.0  # 1.5 * 2**23: fp32 round-to-nearest-even bias trick

F32 = mybir.dt.float32
BF16 = mybir.dt.bfloat16
ALU = mybir.AluOpType
AXX = mybir.AxisListType


def _bitlinear(tc, out, sout, xT, wT, wsl):
    nc = tc.nc
    with ExitStack() as ctx:
        const = ctx.enter_context(tc.tile_pool(name="const", bufs=1))
        statp = ctx.enter_context(tc.tile_pool(name="statp", bufs=1))
        dram = ctx.enter_context(tc.tile_pool(name="dram", bufs=1, space="DRAM"))
        stgx = ctx.enter_context(tc.tile_pool(name="stgx", bufs=2))   # f32 [128,4096]
        stgw = ctx.enter_context(tc.tile_pool(name="stgw", bufs=2))   # f32 [128,4096]
        b2p = ctx.enter_context(tc.tile_pool(name="b2p", bufs=1))     # bf16 [128,4096]
        xqTp = ctx.enter_context(tc.tile_pool(name="xqTp", bufs=1))   # 8x 8KB/part
        wqTp = ctx.enter_context(tc.tile_pool(name="wqTp", bufs=2))   # 4x 8KB/part x2
        psum = ctx.enter_context(tc.tile_pool(name="psum", bufs=1, space="PSUM"))
        osb = ctx.enter_context(tc.tile_pool(name="osb", bufs=2))     # f32 [128,512]

        # ---- Phase 1: local stats ----
        # x-max pass reads the TRANSPOSED shard so the last two group tiles
        # are still resident in the staging slots when sx arrives — they
        # quantize without any reload (max is partition-independent)
        xmax_part = statp.tile([P, 8], F32)
        stat_tiles = {}
        for g in range(8):
            xt = stgx.tile([P, GF], F32, tag="xstage")
            src = xT[g * 512 : (g + 1) * 512, :].rearrange("(c p) t -> p c t", p=P)
            nc.sync.dma_start(xt[:].rearrange("p (c t) -> p c t", c=4), src)
            nc.vector.tensor_reduce(
                xmax_part[:, g : g + 1], xt[:], axis=AXX.X, op=ALU.max,
                apply_absolute_value=True,
            )
            stat_tiles[g] = xt
        wsum_part = statp.tile([P, 4], F32)
        for c in range(4):
            wt = stgw.tile([P, GF], F32, tag="wstage")
            nc.sync.dma_start(wt[:], wsl[_ts(c, P), :])
            nc.vector.tensor_reduce(
                wsum_part[:, c : c + 1], wt[:], axis=AXX.X, op=ALU.add,
                apply_absolute_value=True,
            )
        xmax_c = statp.tile([P, 1], F32)
        nc.vector.tensor_reduce(xmax_c[:], xmax_part[:], axis=AXX.X, op=ALU.max)
        wsum_c = statp.tile([P, 1], F32)
        nc.vector.tensor_reduce(wsum_c[:], wsum_part[:], axis=AXX.X, op=ALU.add)
        xmax_a = statp.tile([P, 1], F32)
        nc.gpsimd.partition_all_reduce(
            xmax_a[:], xmax_c[:], channels=P, reduce_op=bass_isa.ReduceOp.max
        )
        wsum_a = statp.tile([P, 1], F32)
        nc.gpsimd.partition_all_reduce(
            wsum_a[:], wsum_c[:], channels=P, reduce_op=bass_isa.ReduceOp.add
        )

        # ---- one tiny AllGather of [wsum, xmax]; reduce locally (two
        # staggered collectives tested worse: their gpsimd dispatch+exec
        # serialize, so only one could ever be early) ----
        loc = statp.tile([1, 2], F32)
        nc.vector.tensor_copy(loc[0:1, 0:1], wsum_a[0:1, 0:1])
        nc.vector.tensor_copy(loc[0:1, 1:2], xmax_a[0:1, 0:1])
        cin = dram.tile([1, 2], F32)
        cout = dram.tile([1, 2 * NC], F32)
        nc.sync.dma_start(cin[:], loc[:])
        nc.gpsimd.collective_compute(
            "AllGather", ALU.bypass, replica_groups=[list(range(NC))],
            ins=[cin.opt()], outs=[cout.opt()],
        )
        gg = statp.tile([1, 2 * NC], F32)
        nc.sync.dma_start(gg[:], cout[:])
        gg3 = gg[:].rearrange("a (r k) -> a r k", k=2)
        gsum = statp.tile([1, 1], F32)
        nc.vector.tensor_reduce(gsum[:], gg3[:, :, 0:1], axis=AXX.XY, op=ALU.add)
        gmax = statp.tile([1, 1], F32)
        nc.vector.tensor_reduce(gmax[:], gg3[:, :, 1:2], axis=AXX.XY, op=ALU.max)

        thr1 = statp.tile([1, 1], F32)
        nc.vector.tensor_scalar(thr1[:], gsum[:], 0.7 / float(O * I), None, op0=ALU.mult)
        nthr1 = statp.tile([1, 1], F32)
        nc.vector.tensor_scalar(nthr1[:], thr1[:], -1.0, None, op0=ALU.mult)
        thr128 = const.tile([P, 1], F32)
        nc.gpsimd.partition_broadcast(thr128[:], thr1[:])
        nthr128 = const.tile([P, 1], F32)
        nc.gpsimd.partition_broadcast(nthr128[:], nthr1[:])

        gmax_c = statp.tile([1, 1], F32)
        nc.vector.tensor_scalar(gmax_c[:], gmax[:], 1e-12, None, op0=ALU.max)
        rec1 = statp.tile([1, 1], F32)
        nc.vector.reciprocal(rec1[:], gmax_c[:])
        sx1 = statp.tile([1, 1], F32)
        nc.vector.tensor_scalar(sx1[:], rec1[:], 127.0, None, op0=ALU.mult)
        sx128 = const.tile([P, 1], F32)
        nc.gpsimd.partition_broadcast(sx128[:], sx1[:])
        nmagic128 = const.tile([P, 1], F32)
        nc.gpsimd.memset(nmagic128[:], -MAGIC)

        nc.sync.dma_start(sout[0:1, 0:1], gsum[:])
        nc.sync.dma_start(sout[0:1, 1:2], gmax[:])
        nc.sync.dma_start(sout[0:1, 2:3], sx1[:])

        # ---- Phase 2: Xq^T (bf16 [i, t]; 8 group tiles of 4 i-chunks) ----
        # groups 6,7 first: their fp32 tiles are still in the staging slots
        # from the stats pass, so they quantize with zero DMA right at sx
        xq_groups = [None] * 8
        for g in [6, 7, 0, 1, 2, 3, 4, 5]:
            if g >= 6:
                xt = stat_tiles[g]
            else:
                xt = stgx.tile([P, GF], F32, tag="xstage")
                src = xT[g * 512 : (g + 1) * 512, :].rearrange(
                    "(c p) t -> p c t", p=P
                )
                nc.sync.dma_start(xt[:].rearrange("p (c t) -> p c t", c=4), src)
            # u = x*sx + MAGIC computed in place (elementwise same-AP
            # read-write is pipeline-safe; avoids a second staging slot)
            nc.vector.tensor_scalar(
                xt[:], xt[:], sx128[:], MAGIC, op0=ALU.mult, op1=ALU.add
            )
            xg = xqTp.tile([P, GF], BF16, tag=f"xq{g}", name=f"xg{g}")
            # u - MAGIC on the idle ScalarE: Identity(1.0*u + (-MAGIC)) is
            # exact here (the affine step is a single fp32 op whose result is
            # a small integer; the identity spline is exact)
            nc.scalar.activation(
                xg[:], xt[:], mybir.ActivationFunctionType.Identity,
                bias=nmagic128[:], scale=1.0,
            )
            xq_groups[g] = xg

        def lhsT(ic, tb):
            g, c = ic // 4, ic % 4
            base = c * TSH + tb * P
            return xq_groups[g][:, base : base + P]

        # ---- Phase 3: W panels: quantize + count + matmul ----
        qaccs = statp.tile([P, 32], F32)  # sum(Wq) per quarter  ( #pos - #neg )
        naccs = statp.tile([P, 32], F32)  # sum(b2) per quarter  ( #neg )
        for op_ in range(8):  # panels of 512 output columns
            quarters = []
            for q in range(4):  # 8 i-chunks per quarter
                col = op_ * 4 + q
                wt = stgw.tile([P, GF], F32, tag="wstage")
                src = wT[
                    q * 1024 : (q + 1) * 1024, _ts(op_, NMM)
                ].rearrange("(c p) j -> p c j", p=P)
                nc.scalar.dma_start(wt[:].rearrange("p (c j) -> p c j", c=8), src)
                b2 = b2p.tile([P, GF], BF16)
                # op1 doubles as the accum_out reduce op (walrus requires it)
                nc.vector.tensor_scalar(
                    b2[:], wt[:], nthr128[:], None, op0=ALU.is_le, op1=ALU.add,
                    accum_out=naccs[:, col : col + 1],
                )
                wq = wqTp.tile([P, GF], BF16, tag=f"wq{q}")
                nc.vector.scalar_tensor_tensor(
                    wq[:], wt[:], thr128[:], b2[:],
                    op0=ALU.is_ge, op1=ALU.subtract,
                    accum_out=qaccs[:, col : col + 1],
                )
                quarters.append(wq)
            if op_ == 0:
                # ramp-up panel: i-chunk-outer order so every chunk arriving
                # from quantization immediately unlocks 8 matmuls (one per
                # PSUM bank) instead of head-of-line blocking one bank
                ps_tiles = [
                    psum.tile([P, NMM], F32, tag=f"ps{tb}", name=f"ps{tb}")
                    for tb in range(8)
                ]
                for ic in range(32):
                    for tb in range(8):
                        nc.tensor.matmul(
                            ps_tiles[tb][:],
                            lhsT=lhsT(ic, tb),
                            rhs=quarters[ic // 8][:, _ts(ic % 8, NMM)],
                            start=(ic == 0),
                            stop=(ic == 31),
                        )
                for tb in range(8):
                    ot = osb.tile([P, NMM], F32)
                    nc.scalar.copy(ot[:], ps_tiles[tb][:])
                    nc.sync.dma_start(out[_ts(op_ * 8 + tb, P), :], ot[:])
            else:
                for tb in range(8):
                    ps = psum.tile([P, NMM], F32, tag=f"ps{tb}")
                    for ic in range(32):
                        nc.tensor.matmul(
                            ps[:],
                            lhsT=lhsT(ic, tb),
                            rhs=quarters[ic // 8][:, _ts(ic % 8, NMM)],
                            start=(ic == 0),
                            stop=(ic == 31),
                        )
                    ot = osb.tile([P, NMM], F32)
                    nc.scalar.copy(ot[:], ps[:])
                    # chunked output: (panel, tb) tile as one contiguous run
                    nc.sync.dma_start(out[_ts(op_ * 8 + tb, P), :], ot[:])

        # ---- finalize nonzero count: nnz = sum(Wq) + 2*sum(b2) ----
        qacc_c = statp.tile([P, 1], F32)
        nc.vector.tensor_reduce(qacc_c[:], qaccs[:], axis=AXX.X, op=ALU.add)
        nacc_c = statp.tile([P, 1], F32)
        nc.vector.tensor_reduce(nacc_c[:], naccs[:], axis=AXX.X, op=ALU.add)
        nnz_c = statp.tile([P, 1], F32)
        nc.vector.scalar_tensor_tensor(
            nnz_c[:], nacc_c[:], 2.0, qacc_c[:], op0=ALU.mult, op1=ALU.add
        )
        nnz_a = statp.tile([P, 1], F32)
        nc.gpsimd.partition_all_reduce(
            nnz_a[:], nnz_c[:], channels=P, reduce_op=bass_isa.ReduceOp.add
        )
        nc.sync.dma_start(sout[0:1, 3:4], nnz_a[0:1, 0:1])


def _build():
    nc = bacc.Bacc("TRN2", debug=False, enable_asserts=False, num_devices=NC)
    xT_ap = nc.dram_tensor("xT_shard", (I, TSH), F32, kind="ExternalInput").ap()
    wT_ap = nc.dram_tensor("wT_full", (I, O), F32, kind="ExternalInput").ap()
    wsl_ap = nc.dram_tensor("wT_slice", (ISL, O), F32, kind="ExternalInput").ap()
    # chunked layout: row (panel*8 + tb)*128 + r, col c  <->  out[tb*128+r, panel*512+c]
    out_ap = nc.dram_tensor("out_shard", (64 * P, NMM), F32, kind="ExternalOutput").ap()
    st_ap = nc.dram_tensor("stats_out", (1, 4), F32, kind="ExternalOutput").ap()
    with tile.TileContext(nc) as tc:
        _bitlinear(tc, out_ap, st_ap, xT_ap, wT_ap, wsl_ap)
    nc.compile()
    return nc


_NC_CACHE = None


def _get_nc():
    global _NC_CACHE
    if _NC_CACHE is None:
        _NC_CACHE = _build()
    return _NC_CACHE


def _run(x, weight, **spmd_kwargs):
    x = np.ascontiguousarray(np.asarray(x, dtype=np.float32))
    w = np.asarray(weight, dtype=np.float32)
    assert x.shape == (T, I) and w.shape == (O, I)
    nc = _get_nc()
    wT = np.ascontiguousarray(w.T)  # [I, O]
    in_maps = [
        {
            # per-shard transpose directly (cheaper than x.T then slicing)
            "xT_shard": np.ascontiguousarray(x[k * TSH : (k + 1) * TSH].T),
            "wT_full": wT,
            "wT_slice": wT[k * ISL : (k + 1) * ISL],  # contiguous view
        }
        for k in range(NC)
    ]
    res = run_bass_kernel_spmd(nc, in_maps, core_ids=list(range(NC)), **spmd_kwargs)
    outs = res.results

    st0 = outs[0]["stats_out"][0]
    gsum, sx = float(st0[0]), float(st0[2])
    nnz = float(st0[3])  # every core computed the exact global count

    # replicate the reference's fp32 scalar arithmetic
    f32 = np.float32
    n_el = f32(float(O) * float(I))
    abs_mean = f32(f32(gsum) / n_el)
    non_zero_mean = f32(f32(f32(nnz) / n_el) + f32(1e-8))
    scale_w = f32(abs_mean / non_zero_mean)
    scale = f32(np.float64(scale_w) / np.float64(sx))

    # un-chunk each core's [8 panels][8 tb][128][512] output and stack shards
    out = np.empty((T, O), dtype=np.float32)
    for k in range(NC):
        chunk = outs[k]["out_shard"].reshape(8, 8, P, NMM)
        out[k * TSH : (k + 1) * TSH] = (
            chunk.transpose(1, 2, 0, 3).reshape(TSH, O)
        )
    out *= scale
    return out, res


def kernel(x, weight):
    out, _ = _run(x, weight)
    return out



# revision 2
# speedup vs baseline: 1.3067x; 1.3067x over previous
"""BitLinear fake-quant GEMM on 8 TRN2 NeuronCores.

Reference math:
  abs_mean  = mean(|W|);  thr = 0.7*abs_mean
  Wq        = sign(W) * (|W| >= thr)            (ternary)
  scale_w   = abs_mean / (mean(Wq != 0) + 1e-8)
  sx        = 127 / max(|X|)
  Xq        = round(X * sx)                      (integer valued, |.| <= 127)
  out       = (Xq @ Wq^T) * scale_w / sx

Sharding: data-parallel over tokens (8192/8 = 1024 columns of X^T per core);
W is replicated.  The host hands each core PRE-TRANSPOSED operands (x.T shard
and w.T) so both matmul operands already have the contraction dim
(in_features) on partitions — quantization is elementwise and writes straight
into matmul-ready SBUF layouts; the device performs zero transposes.

GEMM runs in fp8e4m3 DoubleRow mode (2 k-tiles per instruction, 0.5
cycles/row = 4x the bf16 row rate) and stays EXACT via a hi/lo split of the
integer activations:
  Xq = a16 + b,  a16 = 16*round(Xq/16) in {-128..128 step 16},  b in [-8,8]
Both parts and the ternary Wq in {-1,0,1} are exactly representable in
fp8e4m3, and fp32 PSUM accumulation of 8192 products of magnitude <= 128
stays below 2^24, so the integer arithmetic is exact.  Each PSUM tile
accumulates 32 DoubleRow matmuls: 16 k-pairs of a16 + 16 k-pairs of b.

The hi/lo parts are produced with the fp32 round-to-nearest-even MAGIC-add
trick: v2 = x*sx + MAGIC encodes MAGIC + Xq; u1 = v2/16 + (15/16)*MAGIC
encodes MAGIC + round(Xq/16) (scalar engine); a16 = 16*u1 - 16*MAGIC cast to
fp8 (scalar engine); b = (v2 - MAGIC) - a16 (vector STT, fp8 out).

Stats: each core reduces its own x shard and a distinct 512-row slice of W^T;
one AllGather of the two per-core scalars + local reduce replaces the global
mean/max all-reduces.  The nonzero count of Wq falls out of the quantization
pass for free via DVE accum_out side-sums.  The final scalar rescale by
scale_w/sx is applied on the host during the unshard; the device stores the
output in bf16 (0.2% relative, well under tolerance) to halve output DMA.

The per-core output is written tile-chunked ([panel][tblock][128][512]); the
host permutes it back during the gather.
"""

from contextlib import ExitStack

import numpy as np

import concourse.bass as bass
import concourse.bass_isa as bass_isa
import concourse.tile as tile
from concourse import bacc, mybir
from concourse.bass import ts as _ts
from concourse.bass_utils import run_bass_kernel_spmd

P = 128
T, I, O = 8192, 4096, 4096  # tokens, in_features, out_features
NC = 8
TSH = T // NC  # 1024 token columns per core
ISL = I // NC  # 512 wT rows per core for stats
NMM = 512  # matmul moving free dim (one fp32 PSUM bank)
GF = 4096  # streaming tile free size (one [128, 4096] fp32 tile = 2 MB)
MAGIC = 12582912.0  # 1.5 * 2**23: fp32 round-to-nearest-even bias trick

F32 = mybir.dt.float32
BF16 = mybir.dt.bfloat16
FP8 = mybir.dt.float8e4
ALU = mybir.AluOpType
AXX = mybir.AxisListType
DR = mybir.MatmulPerfMode.DoubleRow


def _bitlinear(tc, out, sout, xT, wT, wsl):
    nc = tc.nc
    with ExitStack() as ctx:
        const = ctx.enter_context(tc.tile_pool(name="const", bufs=1))
        statp = ctx.enter_context(tc.tile_pool(name="statp", bufs=1))
        dram = ctx.enter_context(tc.tile_pool(name="dram", bufs=1, space="DRAM"))
        stgx = ctx.enter_context(tc.tile_pool(name="stgx", bufs=2))   # f32 [128,4,1024]
        stgw = ctx.enter_context(tc.tile_pool(name="stgw", bufs=2))   # f32 [128,8,512]
        b2p = ctx.enter_context(tc.tile_pool(name="b2p", bufs=1))     # bf16 [128,8,512]
        u1p = ctx.enter_context(tc.tile_pool(name="u1p", bufs=2))     # f32 [128,4,1024]
        ap8 = ctx.enter_context(tc.tile_pool(name="ap8", bufs=1))     # fp8 hi groups
        bp8 = ctx.enter_context(tc.tile_pool(name="bp8", bufs=1))     # fp8 lo groups
        wqTp = ctx.enter_context(tc.tile_pool(name="wqTp", bufs=2))   # fp8 quarters
        psum = ctx.enter_context(tc.tile_pool(name="psum", bufs=1, space="PSUM"))
        osb = ctx.enter_context(tc.tile_pool(name="osb", bufs=2))     # bf16 [128,512]

        # ---- Phase 1: local stats ----
        # x-max pass reads the TRANSPOSED shard; the last two group tiles
        # stay resident in the staging slots so they quantize with no reload
        xmax_part = statp.tile([P, 8], F32)
        stat_tiles = {}
        for g in range(8):
            xt = stgx.tile([P, 4, TSH], F32, tag="xstage")
            src = xT[g * 512 : (g + 1) * 512, :].rearrange("(c p) t -> p c t", p=P)
            nc.sync.dma_start(xt[:], src)
            nc.vector.tensor_reduce(
                xmax_part[:, g : g + 1], xt[:], axis=AXX.XY, op=ALU.max,
                apply_absolute_value=True,
            )
            stat_tiles[g] = xt
        wsum_part = statp.tile([P, 4], F32)
        for c in range(4):
            wt = stgw.tile([P, 8, NMM], F32, tag="wstage")
            nc.sync.dma_start(
                wt[:], wsl[_ts(c, P), :].rearrange("p (c j) -> p c j", c=8)
            )
            nc.vector.tensor_reduce(
                wsum_part[:, c : c + 1], wt[:], axis=AXX.XY, op=ALU.add,
                apply_absolute_value=True,
            )
        xmax_c = statp.tile([P, 1], F32)
        nc.vector.tensor_reduce(xmax_c[:], xmax_part[:], axis=AXX.X, op=ALU.max)
        wsum_c = statp.tile([P, 1], F32)
        nc.vector.tensor_reduce(wsum_c[:], wsum_part[:], axis=AXX.X, op=ALU.add)
        xmax_a = statp.tile([P, 1], F32)
        nc.gpsimd.partition_all_reduce(
            xmax_a[:], xmax_c[:], channels=P, reduce_op=bass_isa.ReduceOp.max
        )
        wsum_a = statp.tile([P, 1], F32)
        nc.gpsimd.partition_all_reduce(
            wsum_a[:], wsum_c[:], channels=P, reduce_op=bass_isa.ReduceOp.add
        )

        # ---- one tiny AllGather of [wsum, xmax]; reduce locally ----
        loc = statp.tile([1, 2], F32)
        nc.vector.tensor_copy(loc[0:1, 0:1], wsum_a[0:1, 0:1])
        nc.vector.tensor_copy(loc[0:1, 1:2], xmax_a[0:1, 0:1])
        cin = dram.tile([1, 2], F32)
        cout = dram.tile([1, 2 * NC], F32)
        nc.sync.dma_start(cin[:], loc[:])
        nc.gpsimd.collective_compute(
            "AllGather", ALU.bypass, replica_groups=[list(range(NC))],
            ins=[cin.opt()], outs=[cout.opt()],
        )
        gg = statp.tile([1, 2 * NC], F32)
        nc.sync.dma_start(gg[:], cout[:])
        gg3 = gg[:].rearrange("a (r k) -> a r k", k=2)
        gsum = statp.tile([1, 1], F32)
        nc.vector.tensor_reduce(gsum[:], gg3[:, :, 0:1], axis=AXX.XY, op=ALU.add)
        gmax = statp.tile([1, 1], F32)
        nc.vector.tensor_reduce(gmax[:], gg3[:, :, 1:2], axis=AXX.XY, op=ALU.max)

        thr1 = statp.tile([1, 1], F32)
        nc.vector.tensor_scalar(thr1[:], gsum[:], 0.7 / float(O * I), None, op0=ALU.mult)
        nthr1 = statp.tile([1, 1], F32)
        nc.vector.tensor_scalar(nthr1[:], thr1[:], -1.0, None, op0=ALU.mult)
        thr128 = const.tile([P, 1], F32)
        nc.gpsimd.partition_broadcast(thr128[:], thr1[:])
        nthr128 = const.tile([P, 1], F32)
        nc.gpsimd.partition_broadcast(nthr128[:], nthr1[:])

        gmax_c = statp.tile([1, 1], F32)
        nc.vector.tensor_scalar(gmax_c[:], gmax[:], 1e-12, None, op0=ALU.max)
        rec1 = statp.tile([1, 1], F32)
        nc.vector.reciprocal(rec1[:], gmax_c[:])
        sx1 = statp.tile([1, 1], F32)
        nc.vector.tensor_scalar(sx1[:], rec1[:], 127.0, None, op0=ALU.mult)
        sx128 = const.tile([P, 1], F32)
        nc.gpsimd.partition_broadcast(sx128[:], sx1[:])
        # bias tiles for the scalar-engine affine passes
        b15mag = const.tile([P, 1], F32)
        nc.gpsimd.memset(b15mag[:], MAGIC * 15.0 / 16.0)
        nmag16 = const.tile([P, 1], F32)
        nc.gpsimd.memset(nmag16[:], -16.0 * MAGIC)

        nc.sync.dma_start(sout[0:1, 0:1], gsum[:])
        nc.sync.dma_start(sout[0:1, 1:2], gmax[:])
        nc.sync.dma_start(sout[0:1, 2:3], sx1[:])

        # ---- Phase 2: Xq^T hi/lo fp8 (8 group tiles of 4 i-chunks) ----
        # groups 6,7 first: their fp32 tiles are still in the staging slots
        a_groups = [None] * 8
        b_groups = [None] * 8
        for g in [6, 7, 0, 1, 2, 3, 4, 5]:
            if g >= 6:
                xt = stat_tiles[g]
            else:
                xt = stgx.tile([P, 4, TSH], F32, tag="xstage")
                src = xT[g * 512 : (g + 1) * 512, :].rearrange(
                    "(c p) t -> p c t", p=P
                )
                nc.sync.dma_start(xt[:], src)
            # v2 = x*sx + MAGIC in place: encodes MAGIC + Xq
            nc.vector.tensor_scalar(
                xt[:], xt[:], sx128[:], MAGIC, op0=ALU.mult, op1=ALU.add
            )
            # u1 = v2/16 + (15/16)*MAGIC: encodes MAGIC + round(Xq/16)
            u1 = u1p.tile([P, 4, TSH], F32, tag="u1")
            nc.scalar.activation(
                u1[:], xt[:], mybir.ActivationFunctionType.Identity,
                bias=b15mag[:], scale=1.0 / 16.0,
            )
            # a16 = 16*u1 - 16*MAGIC -> fp8 (multiples of 16 in [-128,128])
            ag = ap8.tile([P, 4, TSH], FP8, tag=f"a{g}", name=f"a{g}")
            nc.scalar.activation(
                ag[:], u1[:], mybir.ActivationFunctionType.Identity,
                bias=nmag16[:], scale=16.0,
            )
            # b = (v2 - MAGIC) - a16 -> fp8 (integers in [-8,8])
            bg = bp8.tile([P, 4, TSH], FP8, tag=f"b{g}", name=f"b{g}")
            nc.vector.scalar_tensor_tensor(
                bg[:], xt[:], -MAGIC, ag[:], op0=ALU.add, op1=ALU.subtract
            )
            a_groups[g] = ag
            b_groups[g] = bg

        def lhsT_pair(half, kp, tb):
            # k-pair kp covers i-chunks (2kp, 2kp+1); group g holds chunks 4g..4g+3
            g, c = kp // 2, (kp % 2) * 2
            src = a_groups[g] if half == 0 else b_groups[g]
            return src[:, c : c + 2, tb * P : (tb + 1) * P]

        # ---- Phase 3: W panels: quantize + count + DoubleRow matmul ----
        qaccs = statp.tile([P, 32], F32)  # sum(Wq) per quarter  ( #pos - #neg )
        naccs = statp.tile([P, 32], F32)  # sum(b2) per quarter  ( #neg )
        for op_ in range(8):  # panels of 512 output columns
            quarters = []
            for q in range(4):  # 8 i-chunks per quarter
                col = op_ * 4 + q
                wt = stgw.tile([P, 8, NMM], F32, tag="wstage")
                src = wT[
                    q * 1024 : (q + 1) * 1024, _ts(op_, NMM)
                ].rearrange("(c p) j -> p c j", p=P)
                nc.scalar.dma_start(wt[:], src)
                b2 = b2p.tile([P, 8, NMM], BF16)
                # op1 doubles as the accum_out reduce op (walrus requires it)
                nc.vector.tensor_scalar(
                    b2[:], wt[:], nthr128[:], None, op0=ALU.is_le, op1=ALU.add,
                    accum_out=naccs[:, col : col + 1],
                )
                wq = wqTp.tile([P, 8, NMM], FP8, tag=f"wq{q}")
                nc.vector.scalar_tensor_tensor(
                    wq[:], wt[:], thr128[:], b2[:],
                    op0=ALU.is_ge, op1=ALU.subtract,
                    accum_out=qaccs[:, col : col + 1],
                )
                quarters.append(wq)

            def rhs_pair(kp):
                # k-pair kp covers i-chunks (2kp, 2kp+1); quarter q holds 8 chunks
                q, ci = kp // 4, (kp % 4) * 2
                return quarters[q][:, ci : ci + 2, :]

            if op_ == 0:
                # ramp-up panel: kp-outer order so every quarter arriving
                # from quantization immediately unlocks matmuls on all banks
                ps_tiles = [
                    psum.tile([P, NMM], F32, tag=f"ps{tb}", name=f"ps{tb}")
                    for tb in range(8)
                ]
                for kp in range(16):
                    for half in (0, 1):
                        for tb in range(8):
                            nc.tensor.matmul(
                                ps_tiles[tb][:],
                                lhsT=lhsT_pair(half, kp, tb),
                                rhs=rhs_pair(kp),
                                start=(kp == 0 and half == 0),
                                stop=(kp == 15 and half == 1),
                                perf_mode=DR,
                            )
                for tb in range(8):
                    ot = osb.tile([P, NMM], BF16)
                    nc.scalar.copy(ot[:], ps_tiles[tb][:])
                    nc.sync.dma_start(out[_ts(op_ * 8 + tb, P), :], ot[:])
            else:
                for tb in range(8):
                    ps = psum.tile([P, NMM], F32, tag=f"ps{tb}")
                    for half in (0, 1):
                        for kp in range(16):
                            nc.tensor.matmul(
                                ps[:],
                                lhsT=lhsT_pair(half, kp, tb),
                                rhs=rhs_pair(kp),
                                start=(half == 0 and kp == 0),
                                stop=(half == 1 and kp == 15),
                                perf_mode=DR,
                            )
                    ot = osb.tile([P, NMM], BF16)
                    nc.scalar.copy(ot[:], ps[:])
                    # chunked output: (panel, tb) tile as one contiguous run
                    nc.sync.dma_start(out[_ts(op_ * 8 + tb, P), :], ot[:])

        # ---- finalize nonzero count: nnz = sum(Wq) + 2*sum(b2) ----
        qacc_c = statp.tile([P, 1], F32)
        nc.vector.tensor_reduce(qacc_c[:], qaccs[:], axis=AXX.X, op=ALU.add)
        nacc_c = statp.tile([P, 1], F32)
        nc.vector.tensor_reduce(nacc_c[:], naccs[:], axis=AXX.X, op=ALU.add)
        nnz_c = statp.tile([P, 1], F32)
        nc.vector.scalar_tensor_tensor(
            nnz_c[:], nacc_c[:], 2.0, qacc_c[:], op0=ALU.mult, op1=ALU.add
        )
        nnz_a = statp.tile([P, 1], F32)
        nc.gpsimd.partition_all_reduce(
            nnz_a[:], nnz_c[:], channels=P, reduce_op=bass_isa.ReduceOp.add
        )
        nc.sync.dma_start(sout[0:1, 3:4], nnz_a[0:1, 0:1])


def _build():
    nc = bacc.Bacc("TRN2", debug=False, enable_asserts=False, num_devices=NC)
    xT_ap = nc.dram_tensor("xT_shard", (I, TSH), F32, kind="ExternalInput").ap()
    wT_ap = nc.dram_tensor("wT_full", (I, O), F32, kind="ExternalInput").ap()
    wsl_ap = nc.dram_tensor("wT_slice", (ISL, O), F32, kind="ExternalInput").ap()
    # chunked layout: row (panel*8 + tb)*128 + r, col c  <->  out[tb*128+r, panel*512+c]
    out_ap = nc.dram_tensor("out_shard", (64 * P, NMM), BF16, kind="ExternalOutput").ap()
    st_ap = nc.dram_tensor("stats_out", (1, 4), F32, kind="ExternalOutput").ap()
    with tile.TileContext(nc) as tc:
        _bitlinear(tc, out_ap, st_ap, xT_ap, wT_ap, wsl_ap)
    nc.compile()
    return nc


_NC_CACHE = None


def _get_nc():
    global _NC_CACHE
    if _NC_CACHE is None:
        _NC_CACHE = _build()
    return _NC_CACHE


def _run(x, weight, **spmd_kwargs):
    x = np.ascontiguousarray(np.asarray(x, dtype=np.float32))
    w = np.asarray(weight, dtype=np.float32)
    assert x.shape == (T, I) and w.shape == (O, I)
    nc = _get_nc()
    wT = np.ascontiguousarray(w.T)  # [I, O]
    in_maps = [
        {
            # per-shard transpose directly (cheaper than x.T then slicing)
            "xT_shard": np.ascontiguousarray(x[k * TSH : (k + 1) * TSH].T),
            "wT_full": wT,
            "wT_slice": wT[k * ISL : (k + 1) * ISL],  # contiguous view
        }
        for k in range(NC)
    ]
    res = run_bass_kernel_spmd(nc, in_maps, core_ids=list(range(NC)), **spmd_kwargs)
    outs = res.results

    st0 = outs[0]["stats_out"][0]
    gsum, sx = float(st0[0]), float(st0[2])
    nnz = float(st0[3])  # every core computed the exact global count

    # replicate the reference's fp32 scalar arithmetic
    f32 = np.float32
    n_el = f32(float(O) * float(I))
    abs_mean = f32(f32(gsum) / n_el)
    non_zero_mean = f32(f32(f32(nnz) / n_el) + f32(1e-8))
    scale_w = f32(abs_mean / non_zero_mean)
    scale = f32(np.float64(scale_w) / np.float64(sx))

    # un-chunk each core's [8 panels][8 tb][128][512] output and stack shards
    out = np.empty((T, O), dtype=np.float32)
    for k in range(NC):
        chunk = outs[k]["out_shard"].astype(np.float32).reshape(8, 8, P, NMM)
        out[k * TSH : (k + 1) * TSH] = (
            chunk.transpose(1, 2, 0, 3).reshape(TSH, O)
        )
    out *= scale
    return out, res


def kernel(x, weight):
    out, _ = _run(x, weight)
    return out


# revision 4
# speedup vs baseline: 1.3454x; 1.0297x over previous
"""BitLinear fake-quant GEMM on 8 TRN2 NeuronCores.

Reference math:
  abs_mean  = mean(|W|);  thr = 0.7*abs_mean
  Wq        = sign(W) * (|W| >= thr)            (ternary)
  scale_w   = abs_mean / (mean(Wq != 0) + 1e-8)
  sx        = 127 / max(|X|)
  Xq        = round(X * sx)                      (integer valued, |.| <= 127)
  out       = (Xq @ Wq^T) * scale_w / sx

Sharding: data-parallel over tokens (8192/8 = 1024 columns of X^T per core);
W is replicated.  The host hands each core PRE-TRANSPOSED operands (x.T shard
and w.T) so both matmul operands already have the contraction dim
(in_features) on partitions; the device performs zero transposes.

GEMM runs in fp8e4m3 DoubleRow mode (2 k-tiles per instruction, 0.5
cycles/row) and stays EXACT via a hi/lo split of the integer activations:
  Xq = a16 + b,  a16 = 16*round(Xq/16) in {-128..128 step 16},  b in [-8,8]
Both parts and the ternary Wq in {-1,0,1} are exactly representable in
fp8e4m3, and fp32 PSUM accumulation of 8192 products of magnitude <= 128
stays below 2^24.  Each PSUM tile accumulates 32 DoubleRow matmuls: 16
k-pairs of a16 + 16 k-pairs of b.

Schedule (the point of this file's structure):
  1. wT stats slice is read FIRST and its AllGather dispatched immediately,
     so thr lands at ~45us while the x shard is still streaming in.
  2. W panels 0 and 1 quantize on DVE in the window before sx arrives.
  3. After the x-max AllGather, x quantizes in group order [6,7,0..5]
     (6,7 are still resident from the stats pass) while the PE ramps
     through panels 0+1 with a matching k-pair order [12..15, 0..11],
     pacing PE consumption to DVE production.
  4. Panels 2..7 then stream quarter-by-quarter: DVE quantizes panel p+1
     (25.6us) while PE runs panel p (27.3us) - PE-bound steady state.

The hi/lo parts use the fp32 round-to-nearest-even MAGIC-add trick:
v2 = x*sx + MAGIC encodes MAGIC + Xq (in-place, DVE); u1 = v2/16 +
(15/16)*MAGIC encodes MAGIC + round(Xq/16) (scalar engine); a16 = 16*u1 -
16*MAGIC cast to fp8 (scalar engine); b = (v2 - MAGIC) - a16 (DVE STT).

Stats: each core reduces its own x shard and a distinct 512-row slice of
W^T; two tiny AllGathers (W-sum first, x-max second) + local reduces replace
global all-reduces.  The nonzero count of Wq falls out of the quantization
passes for free via DVE accum_out side-sums.  The final scalar rescale by
scale_w/sx is applied on the host during the unshard; the device stores the
output in bf16 (0.2% relative, well under tolerance) to halve output DMA.

The per-core output is written tile-chunked ([panel][tblock][128][512]); the
host permutes it back during the gather.
"""

from contextlib import ExitStack

import numpy as np

import concourse.bass as bass
import concourse.bass_isa as bass_isa
import concourse.tile as tile
from concourse import bacc, mybir
from concourse.bass import ts as _ts
from concourse.bass_utils import run_bass_kernel_spmd

P = 128
T, I, O = 8192, 4096, 4096  # tokens, in_features, out_features
NC = 8
TSH = T // NC  # 1024 token columns per core
ISL = I // NC  # 512 wT rows per core for stats
NMM = 512  # matmul moving free dim (one fp32 PSUM bank)
MAGIC = 12582912.0  # 1.5 * 2**23: fp32 round-to-nearest-even bias trick

F32 = mybir.dt.float32
BF16 = mybir.dt.bfloat16
FP8 = mybir.dt.float8e4
ALU = mybir.AluOpType
AXX = mybir.AxisListType
DR = mybir.MatmulPerfMode.DoubleRow

# x groups quantize in this order (6,7 stay resident from the stats pass);
# the PE k-pair order matches it so the GEMM ramp consumes groups as they
# are produced.  Group g covers k-pairs (2g, 2g+1).
XG_ORDER = [6, 7, 0, 1, 2, 3, 4, 5]
KP_ORDER = [kp for g in XG_ORDER for kp in (2 * g, 2 * g + 1)]


def _bitlinear(tc, out, sout, xT, wT, wsl):
    nc = tc.nc
    with ExitStack() as ctx:
        const = ctx.enter_context(tc.tile_pool(name="const", bufs=1))
        statp = ctx.enter_context(tc.tile_pool(name="statp", bufs=1))
        dram = ctx.enter_context(tc.tile_pool(name="dram", bufs=1, space="DRAM"))
        stgx = ctx.enter_context(tc.tile_pool(name="stgx", bufs=2))   # f32 [128,4,1024]
        stgw = ctx.enter_context(tc.tile_pool(name="stgw", bufs=2))   # f32 [128,8,512]
        b2p = ctx.enter_context(tc.tile_pool(name="b2p", bufs=1))     # bf16 [128,8,512]
        u1p = ctx.enter_context(tc.tile_pool(name="u1p", bufs=2))     # f32 [128,2,1024]
        ap8 = ctx.enter_context(tc.tile_pool(name="ap8", bufs=1))     # fp8 hi groups
        bp8 = ctx.enter_context(tc.tile_pool(name="bp8", bufs=1))     # fp8 lo groups
        wqTp = ctx.enter_context(tc.tile_pool(name="wqTp", bufs=3))   # fp8 quarters
        psum = ctx.enter_context(tc.tile_pool(name="psum", bufs=1, space="PSUM"))
        osb = ctx.enter_context(tc.tile_pool(name="osb", bufs=2))     # bf16 [128,512]

        # ---- Phase 1a: W stats slice first -> earliest possible thr ----
        wsum_part = statp.tile([P, 4], F32)
        for c in range(4):
            wt = stgw.tile([P, 8, NMM], F32, tag="wstage")
            nc.sync.dma_start(
                wt[:], wsl[_ts(c, P), :].rearrange("p (c j) -> p c j", c=8)
            )
            nc.vector.tensor_reduce(
                wsum_part[:, c : c + 1], wt[:], axis=AXX.XY, op=ALU.add,
                apply_absolute_value=True,
            )
        wsum_c = statp.tile([P, 1], F32)
        nc.vector.tensor_reduce(wsum_c[:], wsum_part[:], axis=AXX.X, op=ALU.add)
        wsum_a = statp.tile([P, 1], F32)
        nc.gpsimd.partition_all_reduce(
            wsum_a[:], wsum_c[:], channels=P, reduce_op=bass_isa.ReduceOp.add
        )
        wcin = dram.tile([1, 1], F32)
        wcout = dram.tile([1, NC], F32)
        nc.sync.dma_start(wcin[:], wsum_a[0:1, 0:1])
        nc.gpsimd.collective_compute(
            "AllGather", ALU.bypass, replica_groups=[list(range(NC))],
            ins=[wcin.opt()], outs=[wcout.opt()],
        )
        wgg = statp.tile([1, NC], F32)
        nc.sync.dma_start(wgg[:], wcout[:])
        gsum = statp.tile([1, 1], F32)
        nc.vector.tensor_reduce(gsum[:], wgg[:], axis=AXX.X, op=ALU.add)
        thr1 = statp.tile([1, 1], F32)
        nc.vector.tensor_scalar(thr1[:], gsum[:], 0.7 / float(O * I), None, op0=ALU.mult)
        nthr1 = statp.tile([1, 1], F32)
        nc.vector.tensor_scalar(nthr1[:], thr1[:], -1.0, None, op0=ALU.mult)
        thr128 = const.tile([P, 1], F32)
        nc.gpsimd.partition_broadcast(thr128[:], thr1[:])
        nthr128 = const.tile([P, 1], F32)
        nc.gpsimd.partition_broadcast(nthr128[:], nthr1[:])
        nc.sync.dma_start(sout[0:1, 0:1], gsum[:])

        # ---- Phase 1b: x shard streams in; per-group |x| max ----
        xmax_part = statp.tile([P, 8], F32)
        stat_tiles = {}
        for g in range(8):
            xt = stgx.tile([P, 4, TSH], F32, tag="xstage")
            src = xT[g * 512 : (g + 1) * 512, :].rearrange("(c p) t -> p c t", p=P)
            nc.sync.dma_start(xt[:], src)
            nc.vector.tensor_reduce(
                xmax_part[:, g : g + 1], xt[:], axis=AXX.XY, op=ALU.max,
                apply_absolute_value=True,
            )
            stat_tiles[g] = xt
        xmax_c = statp.tile([P, 1], F32)
        nc.vector.tensor_reduce(xmax_c[:], xmax_part[:], axis=AXX.X, op=ALU.max)
        xmax_a = statp.tile([P, 1], F32)
        nc.gpsimd.partition_all_reduce(
            xmax_a[:], xmax_c[:], channels=P, reduce_op=bass_isa.ReduceOp.max
        )
        xcin = dram.tile([1, 1], F32)
        xcout = dram.tile([1, NC], F32)
        nc.sync.dma_start(xcin[:], xmax_a[0:1, 0:1])
        nc.gpsimd.collective_compute(
            "AllGather", ALU.bypass, replica_groups=[list(range(NC))],
            ins=[xcin.opt()], outs=[xcout.opt()],
        )
        xgg = statp.tile([1, NC], F32)
        nc.sync.dma_start(xgg[:], xcout[:])
        gmax = statp.tile([1, 1], F32)
        nc.vector.tensor_reduce(gmax[:], xgg[:], axis=AXX.X, op=ALU.max)
        gmax_c = statp.tile([1, 1], F32)
        nc.vector.tensor_scalar(gmax_c[:], gmax[:], 1e-12, None, op0=ALU.max)
        rec1 = statp.tile([1, 1], F32)
        nc.vector.reciprocal(rec1[:], gmax_c[:])
        sx1 = statp.tile([1, 1], F32)
        nc.vector.tensor_scalar(sx1[:], rec1[:], 127.0, None, op0=ALU.mult)
        sx128 = const.tile([P, 1], F32)
        nc.gpsimd.partition_broadcast(sx128[:], sx1[:])
        b15mag = const.tile([P, 1], F32)
        nc.gpsimd.memset(b15mag[:], MAGIC * 15.0 / 16.0)
        nmag16 = const.tile([P, 1], F32)
        nc.gpsimd.memset(nmag16[:], -16.0 * MAGIC)
        nc.sync.dma_start(sout[0:1, 1:2], gmax[:])
        nc.sync.dma_start(sout[0:1, 2:3], sx1[:])

        # ---- W panel machinery ----
        qaccs = statp.tile([P, 32], F32)  # sum(Wq) per quarter  ( #pos - #neg )
        naccs = statp.tile([P, 32], F32)  # sum(b2) per quarter  ( #neg )

        def quant_panel(op_):
            quarters = []
            for q in range(4):  # 8 i-chunks per quarter
                col = op_ * 4 + q
                wt = stgw.tile([P, 8, NMM], F32, tag="wstage")
                src = wT[
                    q * 1024 : (q + 1) * 1024, _ts(op_, NMM)
                ].rearrange("(c p) j -> p c j", p=P)
                nc.scalar.dma_start(wt[:], src)
                b2 = b2p.tile([P, 8, NMM], BF16)
                # op1 doubles as the accum_out reduce op (walrus requires it)
                nc.vector.tensor_scalar(
                    b2[:], wt[:], nthr128[:], None, op0=ALU.is_le, op1=ALU.add,
                    accum_out=naccs[:, col : col + 1],
                )
                wq = wqTp.tile([P, 8, NMM], FP8, tag=f"wq{q}")
                nc.vector.scalar_tensor_tensor(
                    wq[:], wt[:], thr128[:], b2[:],
                    op0=ALU.is_ge, op1=ALU.subtract,
                    accum_out=qaccs[:, col : col + 1],
                )
                quarters.append(wq)
            return quarters

        def rhs_pair(quarters, kp):
            q, ci = kp // 4, (kp % 4) * 2
            return quarters[q][:, ci : ci + 2, :]

        # panels 0 and 1 quantize before sx exists (DVE window ~45..95us)
        panel_q = {0: quant_panel(0), 1: quant_panel(1)}

        # ---- Phase 2: Xq^T hi/lo fp8 (order matches KP_ORDER) ----
        a_groups = [None] * 8
        b_groups = [None] * 8
        for g in XG_ORDER:
            if g >= 6:
                xt = stat_tiles[g]
            else:
                xt = stgx.tile([P, 4, TSH], F32, tag="xstage")
                src = xT[g * 512 : (g + 1) * 512, :].rearrange(
                    "(c p) t -> p c t", p=P
                )
                nc.sync.dma_start(xt[:], src)
            # v2 = x*sx + MAGIC in place: encodes MAGIC + Xq
            nc.vector.tensor_scalar(
                xt[:], xt[:], sx128[:], MAGIC, op0=ALU.mult, op1=ALU.add
            )
            ag = ap8.tile([P, 4, TSH], FP8, tag=f"a{g}", name=f"a{g}")
            for h in range(2):  # half-group u1 scratch halves SBUF pressure
                u1 = u1p.tile([P, 2, TSH], F32, tag="u1")
                # u1 = v2/16 + (15/16)*MAGIC: encodes MAGIC + round(Xq/16)
                nc.scalar.activation(
                    u1[:], xt[:, 2 * h : 2 * h + 2, :],
                    mybir.ActivationFunctionType.Identity,
                    bias=b15mag[:], scale=1.0 / 16.0,
                )
                # a16 = 16*u1 - 16*MAGIC -> fp8 (multiples of 16 in [-128,128])
                nc.scalar.activation(
                    ag[:, 2 * h : 2 * h + 2, :], u1[:],
                    mybir.ActivationFunctionType.Identity,
                    bias=nmag16[:], scale=16.0,
                )
            # b = (v2 - MAGIC) - a16 -> fp8 (integers in [-8,8])
            bg = bp8.tile([P, 4, TSH], FP8, tag=f"b{g}", name=f"b{g}")
            nc.vector.scalar_tensor_tensor(
                bg[:], xt[:], -MAGIC, ag[:], op0=ALU.add, op1=ALU.subtract
            )
            a_groups[g] = ag
            b_groups[g] = bg

        def lhsT_pair(half, kp, tb):
            g, c = kp // 2, (kp % 2) * 2
            src = a_groups[g] if half == 0 else b_groups[g]
            return src[:, c : c + 2, tb * P : (tb + 1) * P]

        # ---- Phase 3: remaining W panels quantize (DVE) ----
        for op_ in range(2, 8):
            panel_q[op_] = quant_panel(op_)

        # ---- Phase 4: DoubleRow GEMM ----
        def run_tile(ps, quarters, tb):
            for i, kp in enumerate(KP_ORDER):
                for half in (0, 1):
                    nc.tensor.matmul(
                        ps[:],
                        lhsT=lhsT_pair(half, kp, tb),
                        rhs=rhs_pair(quarters, kp),
                        start=(i == 0 and half == 0),
                        stop=(i == 15 and half == 1),
                        perf_mode=DR,
                    )

        def emit_store(op_, tb, ps):
            ot = osb.tile([P, NMM], BF16)
            nc.scalar.copy(ot[:], ps[:])
            # chunked output: (panel, tb) tile as one contiguous run
            nc.sync.dma_start(out[_ts(op_ * 8 + tb, P), :], ot[:])

        # ramp: panels 0+1 interleaved in two half-tb phases, kp-outer so PE
        # consumption tracks the x-quant production order
        for phase in range(2):
            tbs = range(4 * phase, 4 * phase + 4)
            ps_tiles = {
                (pa, tb): psum.tile(
                    [P, NMM], F32, tag=f"ps{pa}{tb % 4}", name=f"ps_{pa}_{tb}"
                )
                for pa in (0, 1) for tb in tbs
            }
            for i, kp in enumerate(KP_ORDER):
                for half in (0, 1):
                    for pa in (0, 1):
                        for tb in tbs:
                            nc.tensor.matmul(
                                ps_tiles[(pa, tb)][:],
                                lhsT=lhsT_pair(half, kp, tb),
                                rhs=rhs_pair(panel_q[pa], kp),
                                start=(i == 0 and half == 0),
                                stop=(i == 15 and half == 1),
                                perf_mode=DR,
                            )
            for (pa, tb), ps in ps_tiles.items():
                emit_store(pa, tb, ps)

        # steady state: panel p on PE while panel p+1 quantizes on DVE
        for op_ in range(2, 8):
            for tb in range(8):
                ps = psum.tile([P, NMM], F32, tag=f"ps{0 if tb < 4 else 1}{tb % 4}")
                run_tile(ps, panel_q[op_], tb)
                emit_store(op_, tb, ps)

        # ---- finalize nonzero count: nnz = sum(Wq) + 2*sum(b2) ----
        qacc_c = statp.tile([P, 1], F32)
        nc.vector.tensor_reduce(qacc_c[:], qaccs[:], axis=AXX.X, op=ALU.add)
        nacc_c = statp.tile([P, 1], F32)
        nc.vector.tensor_reduce(nacc_c[:], naccs[:], axis=AXX.X, op=ALU.add)
        nnz_c = statp.tile([P, 1], F32)
        nc.vector.scalar_tensor_tensor(
            nnz_c[:], nacc_c[:], 2.0, qacc_c[:], op0=ALU.mult, op1=ALU.add
        )
        nnz_a = statp.tile([P, 1], F32)
        nc.gpsimd.partition_all_reduce(
            nnz_a[:], nnz_c[:], channels=P, reduce_op=bass_isa.ReduceOp.add
        )
        nc.sync.dma_start(sout[0:1, 3:4], nnz_a[0:1, 0:1])


def _build():
    nc = bacc.Bacc("TRN2", debug=False, enable_asserts=False, num_devices=NC)
    xT_ap = nc.dram_tensor("xT_shard", (I, TSH), F32, kind="ExternalInput").ap()
    wT_ap = nc.dram_tensor("wT_full", (I, O), F32, kind="ExternalInput").ap()
    wsl_ap = nc.dram_tensor("wT_slice", (ISL, O), F32, kind="ExternalInput").ap()
    # chunked layout: row (panel*8 + tb)*128 + r, col c  <->  out[tb*128+r, panel*512+c]
    out_ap = nc.dram_tensor("out_shard", (64 * P, NMM), BF16, kind="ExternalOutput").ap()
    st_ap = nc.dram_tensor("stats_out", (1, 4), F32, kind="ExternalOutput").ap()
    with tile.TileContext(nc) as tc:
        _bitlinear(tc, out_ap, st_ap, xT_ap, wT_ap, wsl_ap)
    nc.compile()
    return nc


_NC_CACHE = None


def _get_nc():
    global _NC_CACHE
    if _NC_CACHE is None:
        _NC_CACHE = _build()
    return _NC_CACHE


def _run(x, weight, **spmd_kwargs):
    x = np.ascontiguousarray(np.asarray(x, dtype=np.float32))
    w = np.asarray(weight, dtype=np.float32)
    assert x.shape == (T, I) and w.shape == (O, I)
    nc = _get_nc()
    wT = np.ascontiguousarray(w.T)  # [I, O]
    in_maps = [
        {
            # per-shard transpose directly (cheaper than x.T then slicing)
            "xT_shard": np.ascontiguousarray(x[k * TSH : (k + 1) * TSH].T),
            "wT_full": wT,
            "wT_slice": wT[k * ISL : (k + 1) * ISL],  # contiguous view
        }
        for k in range(NC)
    ]
    res = run_bass_kernel_spmd(nc, in_maps, core_ids=list(range(NC)), **spmd_kwargs)
    outs = res.results

    st0 = outs[0]["stats_out"][0]
    gsum, sx = float(st0[0]), float(st0[2])
    nnz = float(st0[3])  # every core computed the exact global count

    # replicate the reference's fp32 scalar arithmetic
    f32 = np.float32
    n_el = f32(float(O) * float(I))
    abs_mean = f32(f32(gsum) / n_el)
    non_zero_mean = f32(f32(f32(nnz) / n_el) + f32(1e-8))
    scale_w = f32(abs_mean / non_zero_mean)
    scale = f32(np.float64(scale_w) / np.float64(sx))

    # un-chunk each core's [8 panels][8 tb][128][512] output and stack shards
    out = np.empty((T, O), dtype=np.float32)
    for k in range(NC):
        chunk = outs[k]["out_shard"].astype(np.float32).reshape(8, 8, P, NMM)
        out[k * TSH : (k + 1) * TSH] = (
            chunk.transpose(1, 2, 0, 3).reshape(TSH, O)
        )
    out *= scale
    return out, res


def kernel(x, weight):
    out, _ = _run(x, weight)
    return out


# revision 5
# speedup vs baseline: 1.3710x; 1.0190x over previous
"""BitLinear fake-quant GEMM on 8 TRN2 NeuronCores.

Reference math:
  abs_mean  = mean(|W|);  thr = 0.7*abs_mean
  Wq        = sign(W) * (|W| >= thr)            (ternary)
  scale_w   = abs_mean / (mean(Wq != 0) + 1e-8)
  sx        = 127 / max(|X|)
  Xq        = round(X * sx)                      (integer valued, |.| <= 127)
  out       = (Xq @ Wq^T) * scale_w / sx

Sharding: data-parallel over tokens (8192/8 = 1024 columns of X^T per core);
W is replicated.  The host hands each core PRE-TRANSPOSED operands (x.T shard
and w.T) so both matmul operands already have the contraction dim
(in_features) on partitions; the device performs zero transposes.

GEMM runs in fp8e4m3 DoubleRow mode (2 k-tiles per instruction, 0.5
cycles/row) and stays EXACT via a hi/lo split of the integer activations:
  Xq = a16 + b,  a16 = 16*round(Xq/16) in {-128..128 step 16},  b in [-8,8]
Both parts and the ternary Wq in {-1,0,1} are exactly representable in
fp8e4m3, and fp32 PSUM accumulation of 8192 products of magnitude <= 128
stays below 2^24.  Each PSUM tile accumulates 32 DoubleRow matmuls: 16
k-pairs of a16 + 16 k-pairs of b.

Schedule (the point of this file's structure):
  1. wT stats slice is read FIRST and its AllGather dispatched immediately,
     so thr lands at ~45us while the x shard is still streaming in.
  2. W panels 0 and 1 quantize on DVE in the window before sx arrives.
  3. After the x-max AllGather, x quantizes in group order [6,7,0..5]
     (6,7 are still resident from the stats pass) while the PE ramps
     through panels 0+1 with a matching k-pair order [12..15, 0..11],
     pacing PE consumption to DVE production.
  4. Panels 2..7 then stream quarter-by-quarter: DVE quantizes panel p+1
     (25.6us) while PE runs panel p (27.3us) - PE-bound steady state.

The hi/lo parts use the fp32 round-to-nearest-even MAGIC-add trick:
v2 = x*sx + MAGIC encodes MAGIC + Xq (in-place, DVE); u1 = v2/16 +
(15/16)*MAGIC encodes MAGIC + round(Xq/16) (scalar engine); a16 = 16*u1 -
16*MAGIC cast to fp8 (scalar engine); b = (v2 - MAGIC) - a16 (DVE STT).

Stats: each core reduces its own x shard and a distinct 512-row slice of
W^T; two tiny AllGathers (W-sum first, x-max second) + local reduces replace
global all-reduces.  The nonzero count of Wq falls out of the quantization
passes for free via DVE accum_out side-sums.  The final scalar rescale by
scale_w/sx is applied on the host during the unshard; the device stores the
output in bf16 (0.2% relative, well under tolerance) to halve output DMA.

The per-core output is written tile-chunked ([panel][tblock][128][512]); the
host permutes it back during the gather.
"""

from contextlib import ExitStack

import numpy as np

import concourse.bass as bass
import concourse.bass_isa as bass_isa
import concourse.tile as tile
from concourse import bacc, mybir
from concourse.bass import ts as _ts
from concourse.bass_utils import run_bass_kernel_spmd

P = 128
T, I, O = 8192, 4096, 4096  # tokens, in_features, out_features
NC = 8
TSH = T // NC  # 1024 token columns per core
ISL = I // NC  # 512 wT rows per core for stats
NMM = 512  # matmul moving free dim (one fp32 PSUM bank)
MAGIC = 12582912.0  # 1.5 * 2**23: fp32 round-to-nearest-even bias trick

F32 = mybir.dt.float32
BF16 = mybir.dt.bfloat16
FP8 = mybir.dt.float8e4
ALU = mybir.AluOpType
AXX = mybir.AxisListType
DR = mybir.MatmulPerfMode.DoubleRow

# x groups quantize in this order (6,7 stay resident from the stats pass);
# the PE k-pair order matches it so the GEMM ramp consumes groups as they
# are produced.  Group g covers k-pairs (2g, 2g+1).
XG_ORDER = [6, 7, 0, 1, 2, 3, 4, 5]
KP_ORDER = [kp for g in XG_ORDER for kp in (2 * g, 2 * g + 1)]


def _bitlinear(tc, out, sout, xT, wT, wsl):
    nc = tc.nc
    with ExitStack() as ctx:
        const = ctx.enter_context(tc.tile_pool(name="const", bufs=1))
        statp = ctx.enter_context(tc.tile_pool(name="statp", bufs=1))
        dram = ctx.enter_context(tc.tile_pool(name="dram", bufs=1, space="DRAM"))
        stgx = ctx.enter_context(tc.tile_pool(name="stgx", bufs=2))   # f32 [128,4,1024]
        stgw = ctx.enter_context(tc.tile_pool(name="stgw", bufs=2))   # f32 [128,8,512]
        b2p = ctx.enter_context(tc.tile_pool(name="b2p", bufs=1))     # bf16 [128,8,512]
        u1p = ctx.enter_context(tc.tile_pool(name="u1p", bufs=2))     # f32 [128,2,1024]
        ap8 = ctx.enter_context(tc.tile_pool(name="ap8", bufs=1))     # fp8 hi groups
        bp8 = ctx.enter_context(tc.tile_pool(name="bp8", bufs=1))     # fp8 lo groups
        wqTp = ctx.enter_context(tc.tile_pool(name="wqTp", bufs=3))   # fp8 quarters
        psum = ctx.enter_context(tc.tile_pool(name="psum", bufs=1, space="PSUM"))
        osb = ctx.enter_context(tc.tile_pool(name="osb", bufs=2))     # bf16 [128,512]

        # constants first: Pool SEQ must not be frozen behind collective waits
        b15mag = const.tile([P, 1], F32)
        nc.gpsimd.memset(b15mag[:], MAGIC * 15.0 / 16.0)
        nmag16 = const.tile([P, 1], F32)
        nc.gpsimd.memset(nmag16[:], -16.0 * MAGIC)

        # ---- Phase 1a: W stats slice first -> earliest possible thr ----
        wsum_part = statp.tile([P, 4], F32)
        for c in range(4):
            wt = stgw.tile([P, 8, NMM], F32, tag="wstage")
            nc.sync.dma_start(
                wt[:], wsl[_ts(c, P), :].rearrange("p (c j) -> p c j", c=8)
            )
            nc.vector.tensor_reduce(
                wsum_part[:, c : c + 1], wt[:], axis=AXX.XY, op=ALU.add,
                apply_absolute_value=True,
            )
        wsum_c = statp.tile([P, 1], F32)
        nc.vector.tensor_reduce(wsum_c[:], wsum_part[:], axis=AXX.X, op=ALU.add)
        wsum_a = statp.tile([P, 1], F32)
        nc.gpsimd.partition_all_reduce(
            wsum_a[:], wsum_c[:], channels=P, reduce_op=bass_isa.ReduceOp.add
        )
        wcin = dram.tile([1, 1], F32)
        wcout = dram.tile([1, NC], F32)
        nc.gpsimd.dma_start(wcin[:], wsum_a[0:1, 0:1])
        nc.gpsimd.collective_compute(
            "AllGather", ALU.bypass, replica_groups=[list(range(NC))],
            ins=[wcin.opt()], outs=[wcout.opt()],
        )
        wgg = statp.tile([1, NC], F32)
        nc.gpsimd.dma_start(wgg[:], wcout[:])
        gsum = statp.tile([1, 1], F32)
        nc.vector.tensor_reduce(gsum[:], wgg[:], axis=AXX.X, op=ALU.add)
        thr1 = statp.tile([1, 1], F32)
        nc.vector.tensor_scalar(thr1[:], gsum[:], 0.7 / float(O * I), None, op0=ALU.mult)
        nthr1 = statp.tile([1, 1], F32)
        nc.vector.tensor_scalar(nthr1[:], thr1[:], -1.0, None, op0=ALU.mult)
        thr128 = const.tile([P, 1], F32)
        nc.gpsimd.partition_broadcast(thr128[:], thr1[:])
        nthr128 = const.tile([P, 1], F32)
        nc.gpsimd.partition_broadcast(nthr128[:], nthr1[:])
        nc.sync.dma_start(sout[0:1, 0:1], gsum[:])

        # ---- Phase 1b: x shard streams in; per-group |x| max ----
        xmax_part = statp.tile([P, 8], F32)
        stat_tiles = {}
        for g in range(8):
            xt = stgx.tile([P, 4, TSH], F32, tag="xstage")
            src = xT[g * 512 : (g + 1) * 512, :].rearrange("(c p) t -> p c t", p=P)
            nc.sync.dma_start(xt[:], src)
            nc.vector.tensor_reduce(
                xmax_part[:, g : g + 1], xt[:], axis=AXX.XY, op=ALU.max,
                apply_absolute_value=True,
            )
            stat_tiles[g] = xt
        xmax_c = statp.tile([P, 1], F32)
        nc.vector.tensor_reduce(xmax_c[:], xmax_part[:], axis=AXX.X, op=ALU.max)
        xmax_a = statp.tile([P, 1], F32)
        nc.gpsimd.partition_all_reduce(
            xmax_a[:], xmax_c[:], channels=P, reduce_op=bass_isa.ReduceOp.max
        )
        xcin = dram.tile([1, 1], F32)
        xcout = dram.tile([1, NC], F32)
        nc.gpsimd.dma_start(xcin[:], xmax_a[0:1, 0:1])
        nc.gpsimd.collective_compute(
            "AllGather", ALU.bypass, replica_groups=[list(range(NC))],
            ins=[xcin.opt()], outs=[xcout.opt()],
        )
        xgg = statp.tile([1, NC], F32)
        nc.gpsimd.dma_start(xgg[:], xcout[:])
        gmax = statp.tile([1, 1], F32)
        nc.vector.tensor_reduce(gmax[:], xgg[:], axis=AXX.X, op=ALU.max)
        gmax_c = statp.tile([1, 1], F32)
        nc.vector.tensor_scalar(gmax_c[:], gmax[:], 1e-12, None, op0=ALU.max)
        rec1 = statp.tile([1, 1], F32)
        nc.vector.reciprocal(rec1[:], gmax_c[:])
        sx1 = statp.tile([1, 1], F32)
        nc.vector.tensor_scalar(sx1[:], rec1[:], 127.0, None, op0=ALU.mult)
        sx128 = const.tile([P, 1], F32)
        nc.gpsimd.partition_broadcast(sx128[:], sx1[:])
        nc.sync.dma_start(sout[0:1, 1:2], gmax[:])
        nc.sync.dma_start(sout[0:1, 2:3], sx1[:])

        # ---- W panel machinery ----
        qaccs = statp.tile([P, 32], F32)  # sum(Wq) per quarter  ( #pos - #neg )
        naccs = statp.tile([P, 32], F32)  # sum(b2) per quarter  ( #neg )

        def quant_panel(op_, queue="scalar"):
            quarters = [None] * 4
            for q in (3, 0, 1, 2):  # KP_ORDER starts at kp 12 -> quarter 3
                col = op_ * 4 + q
                wt = stgw.tile([P, 8, NMM], F32, tag="wstage")
                src = wT[
                    q * 1024 : (q + 1) * 1024, _ts(op_, NMM)
                ].rearrange("(c p) j -> p c j", p=P)
                getattr(nc, queue).dma_start(wt[:], src)
                b2 = b2p.tile([P, 8, NMM], BF16)
                # op1 doubles as the accum_out reduce op (walrus requires it)
                nc.vector.tensor_scalar(
                    b2[:], wt[:], nthr128[:], None, op0=ALU.is_le, op1=ALU.add,
                    accum_out=naccs[:, col : col + 1],
                )
                wq = wqTp.tile([P, 8, NMM], FP8, tag=f"wq{q}")
                nc.vector.scalar_tensor_tensor(
                    wq[:], wt[:], thr128[:], b2[:],
                    op0=ALU.is_ge, op1=ALU.subtract,
                    accum_out=qaccs[:, col : col + 1],
                )
                quarters[q] = wq
            return quarters

        def rhs_pair(quarters, kp):
            q, ci = kp // 4, (kp % 4) * 2
            return quarters[q][:, ci : ci + 2, :]

        # panel 0 quantizes before sx exists (DVE window after the x reduces)
        panel_q = {0: quant_panel(0, queue="sync")}

        # ---- Phase 2: Xq^T hi/lo fp8 (order matches KP_ORDER) ----
        a_groups = [None] * 8
        b_groups = [None] * 8
        for g in XG_ORDER:
            if g >= 6:
                xt = stat_tiles[g]
            else:
                xt = stgx.tile([P, 4, TSH], F32, tag="xstage")
                src = xT[g * 512 : (g + 1) * 512, :].rearrange(
                    "(c p) t -> p c t", p=P
                )
                nc.sync.dma_start(xt[:], src)
            # v2 = x*sx + MAGIC in place: encodes MAGIC + Xq
            nc.vector.tensor_scalar(
                xt[:], xt[:], sx128[:], MAGIC, op0=ALU.mult, op1=ALU.add
            )
            ag = ap8.tile([P, 4, TSH], FP8, tag=f"a{g}", name=f"a{g}")
            for h in range(2):  # half-group u1 scratch halves SBUF pressure
                u1 = u1p.tile([P, 2, TSH], F32, tag="u1")
                # u1 = v2/16 + (15/16)*MAGIC: encodes MAGIC + round(Xq/16)
                nc.scalar.activation(
                    u1[:], xt[:, 2 * h : 2 * h + 2, :],
                    mybir.ActivationFunctionType.Identity,
                    bias=b15mag[:], scale=1.0 / 16.0,
                )
                # a16 = 16*u1 - 16*MAGIC -> fp8 (multiples of 16 in [-128,128])
                nc.scalar.activation(
                    ag[:, 2 * h : 2 * h + 2, :], u1[:],
                    mybir.ActivationFunctionType.Identity,
                    bias=nmag16[:], scale=16.0,
                )
            # b = (v2 - MAGIC) - a16 -> fp8 (integers in [-8,8])
            bg = bp8.tile([P, 4, TSH], FP8, tag=f"b{g}", name=f"b{g}")
            nc.vector.scalar_tensor_tensor(
                bg[:], xt[:], -MAGIC, ag[:], op0=ALU.add, op1=ALU.subtract
            )
            a_groups[g] = ag
            b_groups[g] = bg

        def lhsT_pair(half, kp, tb):
            g, c = kp // 2, (kp % 2) * 2
            src = a_groups[g] if half == 0 else b_groups[g]
            return src[:, c : c + 2, tb * P : (tb + 1) * P]

        # ---- Phase 3: remaining W panels quantize (DVE) ----
        for op_ in range(1, 8):
            panel_q[op_] = quant_panel(op_)

        # ---- Phase 4: DoubleRow GEMM ----
        def run_tile(ps, quarters, tb):
            for i, kp in enumerate(KP_ORDER):
                for half in (0, 1):
                    nc.tensor.matmul(
                        ps[:],
                        lhsT=lhsT_pair(half, kp, tb),
                        rhs=rhs_pair(quarters, kp),
                        start=(i == 0 and half == 0),
                        stop=(i == 15 and half == 1),
                        perf_mode=DR,
                    )

        def emit_store(op_, tb, ps):
            ot = osb.tile([P, NMM], BF16)
            nc.scalar.copy(ot[:], ps[:])
            # chunked output: (panel, tb) tile as one contiguous run
            nc.sync.dma_start(out[_ts(op_ * 8 + tb, P), :], ot[:])

        # ramp: panel 0 kp-outer across all 8 banks so PE consumption tracks
        # the x-quant production order group by group
        ps_tiles = [
            psum.tile([P, NMM], F32, tag=f"ps{tb}", name=f"ps_{tb}")
            for tb in range(8)
        ]
        for i, kp in enumerate(KP_ORDER):
            for half in (0, 1):
                for tb in range(8):
                    nc.tensor.matmul(
                        ps_tiles[tb][:],
                        lhsT=lhsT_pair(half, kp, tb),
                        rhs=rhs_pair(panel_q[0], kp),
                        start=(i == 0 and half == 0),
                        stop=(i == 15 and half == 1),
                        perf_mode=DR,
                    )
        for tb in range(8):
            emit_store(0, tb, ps_tiles[tb])

        # steady state: panel p on PE while panel p+1 quantizes on DVE
        for op_ in range(1, 8):
            for tb in range(8):
                ps = psum.tile([P, NMM], F32, tag=f"ps{tb}")
                run_tile(ps, panel_q[op_], tb)
                emit_store(op_, tb, ps)

        # ---- finalize nonzero count: nnz = sum(Wq) + 2*sum(b2) ----
        qacc_c = statp.tile([P, 1], F32)
        nc.vector.tensor_reduce(qacc_c[:], qaccs[:], axis=AXX.X, op=ALU.add)
        nacc_c = statp.tile([P, 1], F32)
        nc.vector.tensor_reduce(nacc_c[:], naccs[:], axis=AXX.X, op=ALU.add)
        nnz_c = statp.tile([P, 1], F32)
        nc.vector.scalar_tensor_tensor(
            nnz_c[:], nacc_c[:], 2.0, qacc_c[:], op0=ALU.mult, op1=ALU.add
        )
        nnz_a = statp.tile([P, 1], F32)
        nc.gpsimd.partition_all_reduce(
            nnz_a[:], nnz_c[:], channels=P, reduce_op=bass_isa.ReduceOp.add
        )
        nc.sync.dma_start(sout[0:1, 3:4], nnz_a[0:1, 0:1])


def _build():
    nc = bacc.Bacc("TRN2", debug=False, enable_asserts=False, num_devices=NC)
    xT_ap = nc.dram_tensor("xT_shard", (I, TSH), F32, kind="ExternalInput").ap()
    wT_ap = nc.dram_tensor("wT_full", (I, O), F32, kind="ExternalInput").ap()
    wsl_ap = nc.dram_tensor("wT_slice", (ISL, O), F32, kind="ExternalInput").ap()
    # chunked layout: row (panel*8 + tb)*128 + r, col c  <->  out[tb*128+r, panel*512+c]
    out_ap = nc.dram_tensor("out_shard", (64 * P, NMM), BF16, kind="ExternalOutput").ap()
    st_ap = nc.dram_tensor("stats_out", (1, 4), F32, kind="ExternalOutput").ap()
    with tile.TileContext(nc) as tc:
        _bitlinear(tc, out_ap, st_ap, xT_ap, wT_ap, wsl_ap)
    nc.compile()
    return nc


_NC_CACHE = None


def _get_nc():
    global _NC_CACHE
    if _NC_CACHE is None:
        _NC_CACHE = _build()
    return _NC_CACHE


def _run(x, weight, **spmd_kwargs):
    x = np.ascontiguousarray(np.asarray(x, dtype=np.float32))
    w = np.asarray(weight, dtype=np.float32)
    assert x.shape == (T, I) and w.shape == (O, I)
    nc = _get_nc()
    wT = np.ascontiguousarray(w.T)  # [I, O]
    in_maps = [
        {
            # per-shard transpose directly (cheaper than x.T then slicing)
            "xT_shard": np.ascontiguousarray(x[k * TSH : (k + 1) * TSH].T),
            "wT_full": wT,
            "wT_slice": wT[k * ISL : (k + 1) * ISL],  # contiguous view
        }
        for k in range(NC)
    ]
    res = run_bass_kernel_spmd(nc, in_maps, core_ids=list(range(NC)), **spmd_kwargs)
    outs = res.results

    st0 = outs[0]["stats_out"][0]
    gsum, sx = float(st0[0]), float(st0[2])
    nnz = float(st0[3])  # every core computed the exact global count

    # replicate the reference's fp32 scalar arithmetic
    f32 = np.float32
    n_el = f32(float(O) * float(I))
    abs_mean = f32(f32(gsum) / n_el)
    non_zero_mean = f32(f32(f32(nnz) / n_el) + f32(1e-8))
    scale_w = f32(abs_mean / non_zero_mean)
    scale = f32(np.float64(scale_w) / np.float64(sx))

    # un-chunk each core's [8 panels][8 tb][128][512] output and stack shards
    out = np.empty((T, O), dtype=np.float32)
    for k in range(NC):
        chunk = outs[k]["out_shard"].astype(np.float32).reshape(8, 8, P, NMM)
        out[k * TSH : (k + 1) * TSH] = (
            chunk.transpose(1, 2, 0, 3).reshape(TSH, O)
        )
    out *= scale
    return out, res


def kernel(x, weight):
    out, _ = _run(x, weight)
    return out
